# revision 7
# baseline (speedup 1.0000x reference)
"""ConvGRU Trainium2 Bass kernel — 1D Winograd F(2,3) along y.

Math: ConvGRU cell, 3 gates (z, r, q): depthwise 3x3 conv (SAME) + pointwise
1x1, weights int8-fake-quantized per-tensor.

Strategy:
  - Data-parallel over batch: 8 images -> 8 NeuronCores.
  - Depthwise+pointwise fused into matmuls over channel chunks; the 3 vertical
    taps are replaced by Winograd F(2,3): per output tile-row pair, 4
    transformed planes V[uy] (built on DVE from row combos) are contracted
    with host-transformed weights (G applied along ky), leaving only the 3
    horizontal taps as free-dim shifts.  PE work drops from 9 to 6 effective
    full-image passes per gate (4 uy planes at half the rows x 3 dx).
  - m[uy] accumulate in PSUM (pairs packed per bank); DVE applies the inverse
    transform (even rows = m0+m1+m2, odd = m1-m2-m3); ScalarE applies
    sigmoid/tanh with the fused fake-quant scale + combined bias.
  - Image layout: [128, 66 rows x 132 cols] bf16, 1 row pad top/bottom,
    2 col pad left/right (keeps 4B alignment for DVE 2x mode).
"""

import sys

sys.path.insert(0, "/opt/trn_rl_repo")

import ml_dtypes
import numpy as np

HID, INP, C = 128, 320, 448
B, H, W = 8, 64, 128
WP = 132  # padded row width (2 left, 2 right)
HP = 66   # padded rows
NPIX = H * W
NW = 16   # windows of 2 tile-rows (4 pixel rows) each

# channel chunks on partitions: [0:128)=h/rh, [128:256)=x0, [256:384)=x1, [384:448)=x2
CHUNKS = [(0, 128), (128, 256), (256, 384), (384, 448)]

_CACHE = {}


def _build(loop_reps=None):
    import contextlib

    import concourse.bacc as bacc
    import concourse.tile as tile
    from concourse import mybir

    f32 = mybir.dt.float32
    bf16 = mybir.dt.bfloat16
    AF = mybir.ActivationFunctionType

    nc = bacc.Bacc("TRN2", target_bir_lowering=False, debug=False, num_devices=8)

    h32 = nc.dram_tensor("h32", [HID, NPIX], f32, kind="ExternalInput")
    x32 = nc.dram_tensor("x32", [INP, NPIX], f32, kind="ExternalInput")
    wz = nc.dram_tensor("wz", [12, C, HID], bf16, kind="ExternalInput")
    wr = nc.dram_tensor("wr", [12, C, HID], bf16, kind="ExternalInput")
    wq = nc.dram_tensor("wq", [12, C, HID], bf16, kind="ExternalInput")
    sbt_d = nc.dram_tensor("sbt", [HID, 6], f32, kind="ExternalInput")
    out_d = nc.dram_tensor("out", [HID, NPIX], f32, kind="ExternalOutput")

    with tile.TileContext(nc) as tc:
        with (
            tc.tile_pool(name="big", bufs=1) as big,
            tc.tile_pool(name="wp", bufs=1) as wpool,
            tc.tile_pool(name="stage", bufs=2) as stage,
            tc.tile_pool(name="vpool", bufs=2) as vpool,
            tc.tile_pool(name="win", bufs=2) as win,
            tc.tile_pool(name="psum", bufs=1, space="PSUM") as psum,
            tc.For_i(0, loop_reps, 1) if loop_reps else contextlib.nullcontext(),
        ):
            hp = big.tile([128, HP * WP], bf16, name="hp")
            x0p = big.tile([128, HP * WP], bf16, name="x0p")
            x1p = big.tile([128, HP * WP], bf16, name="x1p")
            x2p = big.tile([64, HP * WP], bf16, name="x2p")
            rhp = big.tile([128, HP * WP], bf16, name="rhp")
            zp = big.tile([128, HP * WP], bf16, name="zp")
            imgs = [hp, x0p, x1p, x2p]

            # zero only the pad regions (top/bottom rows, left/right cols)
            for t_ in (hp, x0p, x1p, x2p, rhp):
                t3 = t_.rearrange("p (r c) -> p r c", c=WP)
                nc.vector.memset(t3[:, 0:1, :], 0.0)
                nc.vector.memset(t3[:, HP - 1 : HP, :], 0.0)
                nc.vector.memset(t3[:, :, 0:2], 0.0)
                nc.vector.memset(t3[:, :, 130:132], 0.0)

            sbt = wpool.tile([128, 6], f32)
            nc.sync.dma_start(out=sbt[:], in_=sbt_d[:])

            # weights: [gate][chunk] -> [kc, 12*128] (t = uy*3 + dxi)
            wd = {"z": wz, "r": wr, "q": wq}
            wt = {}
            for g in ("z", "r", "q"):
                for ci, (c0, c1) in enumerate(CHUNKS):
                    kc = c1 - c0
                    wtile = wpool.tile([kc, 12 * 128], bf16, name=f"w_{g}_{ci}")
                    w3 = wtile.rearrange("p (t c) -> p t c", c=128)
                    wsrc = wd[g].rearrange("t c o -> c t o")
                    nc.sync.dma_start(out=w3[:, :, :], in_=wsrc[c0:c1])
                    wt[(g, ci)] = wtile

            # input load + f32->bf16 convert into padded layouts (ScalarE)
            for ci, (c0, c1) in enumerate(CHUNKS):
                kc = c1 - c0
                dst3 = imgs[ci].rearrange("p (r c) -> p r c", c=WP)
                srcd = h32 if ci == 0 else x32
                off = 0 if ci == 0 else c0 - 128
                for qtr in range(4):
                    st = stage.tile([128, 2048], f32, tag="st", name=f"st{ci}{qtr}")
                    nc.sync.dma_start(
                        out=st[:kc, :],
                        in_=srcd[off : off + kc, qtr * 2048 : (qtr + 1) * 2048],
                    )
                    st3 = st.rearrange("p (r c) -> p r c", c=W)
                    nc.scalar.activation(
                        dst3[:kc, 1 + 16 * qtr : 17 + 16 * qtr, 2:130],
                        st3[:kc, :16, :],
                        AF.Copy,
                    )

            out3 = out_d.rearrange("p (r c) -> p r c", c=W)
            hp3 = hp.rearrange("p (r c) -> p r c", c=WP)
            rhp3 = rhp.rearrange("p (r c) -> p r c", c=WP)
            zp3 = zp.rearrange("p (r c) -> p r c", c=WP)

            def v_transform(dst, src, w_, kc=128):
                """dst: V tile [kc, 4*264]; src: padded img; window w_."""
                s3 = src.rearrange("p (r c) -> p r c", c=WP)
                d4 = dst.rearrange("p (u t c) -> p u t c", t=2, c=WP)

                def rows(i):
                    return s3[:kc, 4 * w_ + i : 4 * w_ + i + 3 : 2, :]

                r0, r1, r2, r3 = rows(0), rows(1), rows(2), rows(3)
                nc.vector.tensor_sub(d4[:kc, 0], r0, r2)
                nc.vector.tensor_add(d4[:kc, 1], r1, r2)
                nc.vector.tensor_sub(d4[:kc, 2], r2, r1)
                nc.vector.tensor_sub(d4[:kc, 3], r1, r3)

            def gate_mms(g, vtiles, mt01, mt23):
                """48 MMs for gate g: m[uy] pairs packed per PSUM bank.

                m layout per bank tile: [uy-in-pair(2), tyl(2), 128] = 512 f32.
                rhs reads V cols 2+dx..130+dx — V pad cols are zero, so SAME
                boundary handling is free (no clamping)."""
                for pair, mt in ((0, mt01), (1, mt23)):
                    for upos, uy in enumerate((2 * pair, 2 * pair + 1)):
                        i = 0
                        for dxi, dx in ((1, 0), (0, -1), (2, 1)):
                            for ci, (c0, c1) in enumerate(CHUNKS):
                                kc = c1 - c0
                                v4 = vtiles[ci].rearrange(
                                    "p (u t c) -> p u t c", t=2, c=WP
                                )
                                nc.tensor.matmul(
                                    mt[:, upos * 256 : (upos + 1) * 256],
                                    wt[(g, ci)][
                                        :,
                                        (uy * 3 + dxi) * 128 : (uy * 3 + dxi + 1) * 128,
                                    ],
                                    v4[:kc, uy, :, 2 + dx : 130 + dx],
                                    start=(i == 0),
                                    stop=(i == 11),
                                )
                                i += 1

            def inverse(mt01, mt23, tagp):
                """Inverse transform -> (Ye, Yo) bf16 [128, 256].

                ScalarE drains m1/m2/m3 to SBUF (TT can't take 2 PSUM
                operands); DVE combines: Ye = m0+m1+m2, Yo = m1-m2-m3."""
                d1 = win.tile([128, 256], bf16, tag=f"iva{tagp}", name=f"iva{tagp}")
                d2 = win.tile([128, 256], bf16, tag=f"ivb{tagp}", name=f"ivb{tagp}")
                d3 = win.tile([128, 256], bf16, tag=f"ivc{tagp}", name=f"ivc{tagp}")
                nc.scalar.activation(d1[:], mt01[:, 256:512], AF.Copy)
                nc.scalar.activation(d2[:], mt23[:, 0:256], AF.Copy)
                nc.scalar.activation(d3[:], mt23[:, 256:512], AF.Copy)
                t0 = win.tile([128, 256], bf16, tag=f"t0{tagp}", name=f"t0{tagp}")
                ye = win.tile([128, 256], bf16, tag=f"ye{tagp}", name=f"ye{tagp}")
                t1 = win.tile([128, 256], bf16, tag=f"t1{tagp}", name=f"t1{tagp}")
                yo = win.tile([128, 256], bf16, tag=f"yo{tagp}", name=f"yo{tagp}")
                nc.vector.tensor_add(t0[:], mt01[:, 0:256], d1[:])
                nc.vector.tensor_add(ye[:], t0[:], d2[:])
                nc.vector.tensor_sub(t1[:], d1[:], d2[:])
                nc.vector.tensor_sub(yo[:], t1[:], d3[:])
                return ye, yo

            def gate_act(ye, yo, d3, w_, func, sc, bi):
                """Write activated rows into padded img data cols 2:130."""
                y2 = ye.rearrange("p (t c) -> p t c", c=W)
                o2 = yo.rearrange("p (t c) -> p t c", c=W)
                nc.scalar.activation(
                    d3[:, 1 + 4 * w_ : 1 + 4 * w_ + 3 : 2, 2:130], y2[:],
                    func, bias=sbt[:, bi : bi + 1], scale=sbt[:, sc : sc + 1],
                )
                nc.scalar.activation(
                    d3[:, 2 + 4 * w_ : 2 + 4 * w_ + 3 : 2, 2:130], o2[:],
                    func, bias=sbt[:, bi : bi + 1], scale=sbt[:, sc : sc + 1],
                )

            vts_prev = None
            for it in range(NW + 1):
                w = it
                wq_ = it - 1
                vts = None
                if w < NW:
                    vts = []
                    for ci, (c0, c1) in enumerate(CHUNKS):
                        vt = vpool.tile(
                            [128 if ci < 3 else 64, 4 * 264], bf16,
                            tag=f"v{ci}", name=f"v{ci}_{w}",
                        )
                        v_transform(vt, imgs[ci], w, kc=c1 - c0)
                        vts.append(vt)
                    zm01 = psum.tile([128, 512], f32, tag="zm01", name=f"zm01_{w}")
                    zm23 = psum.tile([128, 512], f32, tag="zm23", name=f"zm23_{w}")
                    rm01 = psum.tile([128, 512], f32, tag="rm01", name=f"rm01_{w}")
                    rm23 = psum.tile([128, 512], f32, tag="rm23", name=f"rm23_{w}")
                    gate_mms("z", vts, zm01, zm23)
                    gate_mms("r", vts, rm01, rm23)
                    zye, zyo = inverse(zm01, zm23, "z")
                    gate_act(zye, zyo, zp3, w, AF.Sigmoid, 0, 1)
                    rye, ryo = inverse(rm01, rm23, "r")
                    rp = win.tile([128, 4 * W], bf16, tag="rp", name=f"rp{w}")
                    rp3 = rp.rearrange("p (r c) -> p r c", c=W)
                    ry2 = rye.rearrange("p (t c) -> p t c", c=W)
                    ro2 = ryo.rearrange("p (t c) -> p t c", c=W)
                    nc.scalar.activation(
                        rp3[:, 0:3:2, :], ry2[:], AF.Sigmoid,
                        bias=sbt[:, 3:4], scale=sbt[:, 2:3],
                    )
                    nc.scalar.activation(
                        rp3[:, 1:4:2, :], ro2[:], AF.Sigmoid,
                        bias=sbt[:, 3:4], scale=sbt[:, 2:3],
                    )
                    nc.vector.tensor_mul(
                        rhp3[:, 1 + 4 * w : 5 + 4 * w, 2:130], rp3[:, 0:4, :],
                        hp3[:, 1 + 4 * w : 5 + 4 * w, 2:130],
                    )
                if wq_ >= 0:
                    vrh = vpool.tile([128, 4 * 264], bf16, tag="vrh", name=f"vrh{wq_}")
                    v_transform(vrh, rhp, wq_)
                    qvts = [vrh, vts_prev[1], vts_prev[2], vts_prev[3]]
                    qm01 = psum.tile([128, 512], f32, tag="qm01", name=f"qm01_{wq_}")
                    qm23 = psum.tile([128, 512], f32, tag="qm23", name=f"qm23_{wq_}")
                    gate_mms("q", qvts, qm01, qm23)
                    qye, qyo = inverse(qm01, qm23, "q")
                    qp_ = win.tile([128, 4 * W], bf16, tag="qp", name=f"qp{wq_}")
                    qp3 = qp_.rearrange("p (r c) -> p r c", c=W)
                    qy2 = qye.rearrange("p (t c) -> p t c", c=W)
                    qo2 = qyo.rearrange("p (t c) -> p t c", c=W)
                    nc.scalar.activation(
                        qp3[:, 0:3:2, :], qy2[:], AF.Tanh,
                        bias=sbt[:, 5:6], scale=sbt[:, 4:5],
                    )
                    nc.scalar.activation(
                        qp3[:, 1:4:2, :], qo2[:], AF.Tanh,
                        bias=sbt[:, 5:6], scale=sbt[:, 4:5],
                    )
                    # GRU: out = h + z*(q - h)
                    hrows = hp3[:, 1 + 4 * wq_ : 5 + 4 * wq_, 2:130]
                    zrows = zp3[:, 1 + 4 * wq_ : 5 + 4 * wq_, 2:130]
                    d1 = win.tile([128, 4 * W], bf16, tag="d1", name=f"d1{wq_}")
                    d13 = d1.rearrange("p (r c) -> p r c", c=W)
                    nc.vector.tensor_sub(d13[:, 0:4, :], qp3[:, 0:4, :], hrows)
                    d2 = win.tile([128, 4 * W], bf16, tag="d2", name=f"d2{wq_}")
                    d23 = d2.rearrange("p (r c) -> p r c", c=W)
                    nc.vector.tensor_mul(d23[:, 0:4, :], zrows, d13[:, 0:4, :])
                    ow = win.tile([128, 4 * W], f32, tag="ow", name=f"ow{wq_}")
                    ow3 = ow.rearrange("p (r c) -> p r c", c=W)
                    nc.vector.tensor_add(ow3[:, 0:4, :], hrows, d23[:, 0:4, :])
                    nc.sync.dma_start(
                        out=out3[:, 4 * wq_ : 4 * wq_ + 4, :],
                        in_=ow3[:, 0:4, :],
                    )
                vts_prev = vts

    nc.compile()
    return nc


# NOTE: _build above is superseded; see _build2 which is the real one.


def _fq_int(w):
    w = np.asarray(w, np.float32)
    scale = (
        np.maximum(np.max(np.abs(w)), np.float32(1e-8)) / np.float32(127.0)
    ).astype(np.float32)
    q = np.clip(np.round(w / scale), -128, 127).astype(np.float64)
    return q, scale


_G = np.array(
    [[1, 0, 0], [0.5, 0.5, 0.5], [0.5, -0.5, 0.5], [0, 0, 1]], np.float64
)


def _prep_gate(wdg, bdg, wpg, bpg):
    qd, sd = _fq_int(wdg)  # [C,1,3,3]
    qp, sp = _fq_int(wpg)  # [HID,C,1,1]
    qp2 = qp[:, :, 0, 0]  # [HID, C]
    wy = np.einsum("ua,cab->cub", _G, qd[:, 0])  # [C,4,3]
    lhsT = np.empty((12, C, HID), np.float64)
    for uy in range(4):
        for kx in range(3):
            M = qp2 * wy[:, uy, kx][None, :]
            lhsT[uy * 3 + kx] = M.T
    scale = np.float32(sd) * np.float32(sp)
    bias = (
        np.float32(sp) * (qp2 @ np.asarray(bdg, np.float64))
        + np.asarray(bpg, np.float64)
    ).astype(np.float32)
    return lhsT.astype(ml_dtypes.bfloat16), scale, bias


def last_in_maps(inputs):
    h = np.asarray(inputs["h"], np.float32)
    x = np.asarray(inputs["x"], np.float32)

    wz, s_z, b_z = _prep_gate(
        inputs["wdz"], inputs["bdz"], inputs["wpz"], inputs["bpz"]
    )
    wr, s_r, b_r = _prep_gate(
        inputs["wdr"], inputs["bdr"], inputs["wpr"], inputs["bpr"]
    )
    wq, s_q, b_q = _prep_gate(
        inputs["wdq"], inputs["bdq"], inputs["wpq"], inputs["bpq"]
    )

    sbt = np.empty((HID, 6), np.float32)
    sbt[:, 0] = s_z
    sbt[:, 1] = b_z
    sbt[:, 2] = s_r
    sbt[:, 3] = b_r
    sbt[:, 4] = s_q
    sbt[:, 5] = b_q

    in_maps = []
    for i in range(B):
        in_maps.append(
            {
                "h32": np.ascontiguousarray(h[i].reshape(HID, NPIX)),
                "x32": np.ascontiguousarray(x[i].reshape(INP, NPIX)),
                "wz": wz,
                "wr": wr,
                "wq": wq,
                "sbt": sbt,
            }
        )
    return in_maps


def kernel(**inputs):
    from concourse.bass_utils import run_bass_kernel_spmd

    if "nc" not in _CACHE:
        _CACHE["nc"] = _build()
    nc = _CACHE["nc"]

    in_maps = last_in_maps(inputs)

    res = run_bass_kernel_spmd(nc, in_maps, list(range(B)))
    out = np.stack(
        [res.results[i]["out"].reshape(HID, H, W) for i in range(B)], axis=0
    )
    return out.astype(np.float32)
